# revision 24
# baseline (speedup 1.0000x reference)
"""Attention (B=4, S=4096, W=512, E=64) on 8 TRN2 NeuronCores.

Sharding: core c handles batch b = c//2, query half h = c%2 (2048 queries).
Each core receives x[b]^T as bf16 with the key/value columns ordered so that
this core's query half occupies columns [0, 2048) (softmax over keys is
permutation invariant as long as K and V share the order, so odd cores get
the two halves swapped). K/V are computed for the full sequence locally; a
flash-style attention runs over the core's query half. No collectives.

Per-core dataflow (bf16 matmul inputs, fp32 PSUM accumulation):
  x^T streamed over two parallel DMA queues; a burst of warm-up matmuls
  keeps the PE HAM clock-gate at 2.4 GHz through the DMA-paced prologue.
  x^T [512,4096] --[Wv|Wk] pass--> kv = V^T (p0:64) / K^T (p64:128)
  x^T[:, :2048] --[Wq|Wq] pass--> Q^T duplicated on both partition halves
  K^T replicated to partitions 0:64 via SWDGE SBUF->SBUF DMA (row packing)
  scores: S^T[k,q] = K^T.T @ Q^T, two k-tiles packed in PE row groups (e=64)
  P = exp(S^T * 0.125): whole score pairs alternate between ScalarE (LUT
  exp) and VectorE (Schraudolph bit-trick: one tensor_scalar computing
  round(score*A + B) into int16 whose bits ARE the bf16 exp approximation;
  errors average out over the 4096-key softmax). Alternating whole pairs
  amortizes the per-instruction overhead of both engines.
  Z'^T[e+1,q] += V'.T @ P^T with the contraction split into PE row groups:
  keys 0:64 accumulate into zpA (tile T0), keys 64:128 into zpB (tile T8),
  so every matmul in the main loop runs in the same 64x128 PE tiling mode
  (no reconfiguration drains) and the two AV halves execute concurrently.
  V' carries a ones column, so row 64 accumulates the softmax denominator.
  zpA/zpB are evacuated per query chunk (ScalarE copy / VectorE copy) and
  DMA'd out unnormalized; the final o = (zA+zB)[0:64]/(zA+zB)[64] divide
  runs on the host (0.1% of the FLOPs, keeps all 8 PSUM banks for the
  pipeline: 3 double-buffered score pairs + 2 accumulators).
  AV matmuls are emitted three iterations late (carried across query-chunk
  boundaries) so the PE's in-order stream never stalls on an exp.
"""

import numpy as np
import ml_dtypes

import concourse.bass as bass
import concourse.mybir as mybir
import concourse.tile as tile
from concourse import bacc
from concourse.bass import ts
from concourse.masks import make_identity
from concourse.bass_utils import run_bass_kernel_spmd

BF16 = mybir.dt.bfloat16
I16 = mybir.dt.int16
F32 = mybir.dt.float32
NP_BF16 = ml_dtypes.bfloat16

B = 4
S_FULL = 4096
W = 512
E = 64
TQ = 2048  # queries per core
WT = W // 128  # 4 contraction tiles
KT = S_FULL // 128  # 32 key tiles
KP = KT // 2  # 16 key-tile pairs
QC = TQ // 512  # 4 query chunks of 512
NCH = S_FULL // 512  # 8 projection chunks
SCALE = 0.125  # 1/sqrt(E)
# Schraudolph exp for VectorE: bf16(bits = round(s * A + B)) ~= exp(s/8)
SCH_A = 0.125 * 1.4426950408889634 * 128.0  # scale*log2(e)*2^mantissa_bits
SCH_B = 127.0 * 128.0 - 11.0  # exponent bias + error-balancing constant
N_WARM = 40  # PE warm-up transposes during the DMA window
PEND = 3  # AV matmuls trail their score pair by this many iterations

_NC_CACHE = {}


def build_nc():
    nc = bacc.Bacc("TRN2", target_bir_lowering=False)
    xT = nc.dram_tensor("xT", [W, S_FULL], BF16, kind="ExternalInput")
    # weights arrive pre-rearranged to [p, t, m] so the DMA is contiguous
    # (a strided gather here costs ~5us of DMA-land latency)
    wqq = nc.dram_tensor("wqq", [128, WT, 128], BF16, kind="ExternalInput")
    wkv = nc.dram_tensor("wkv", [128, WT, 128], BF16, kind="ExternalInput")
    bqq = nc.dram_tensor("bqq", [128, 1], F32, kind="ExternalInput")
    bkv = nc.dram_tensor("bkv", [128, 1], F32, kind="ExternalInput")
    # unnormalized [V'Z | denom] halves, combined + divided on the host
    y = nc.dram_tensor("y", [2, QC, E + 1, 512], F32, kind="ExternalOutput")

    with tile.TileContext(nc) as tc:
        with (
            tc.tile_pool(name="const", bufs=1) as const,
            tc.tile_pool(name="pp", bufs=8) as ppool,
            tc.tile_pool(name="zsb", bufs=4) as zsbp,
        ):
            # weights/biases first on the scalar HWDGE queue (contiguous)
            wqq_sb = const.tile([128, WT, 128], BF16)
            wkv_sb = const.tile([128, WT, 128], BF16)
            nc.scalar.dma_start(out=wkv_sb, in_=wkv[:, :, :])
            nc.scalar.dma_start(out=wqq_sb, in_=wqq[:, :, :])
            bqq_sb = const.tile([128, 1], F32)
            bkv_sb = const.tile([128, 1], F32)
            nc.scalar.dma_start(out=bkv_sb, in_=bkv[:, :])
            nc.scalar.dma_start(out=bqq_sb, in_=bqq[:, :])

            # x^T: 1024-column blocks on the sync HWDGE queue; blocks 2-3 are
            # emitted just-in-time inside the chunk loop so the per-chunk
            # krep replication DMAs interleave on the same fast queue
            xt_sb = const.tile([128, WT, S_FULL], BF16)

            def emit_xblk(blk):
                for t in range(WT):
                    nc.sync.dma_start(
                        out=xt_sb[:, t, ts(blk, 1024)],
                        in_=xT[t * 128:(t + 1) * 128, ts(blk, 1024)],
                    )

            emit_xblk(0)
            emit_xblk(1)

            ident_bf = const.tile([64, 64], BF16)
            make_identity(nc, ident_bf)

            kv_sb = const.tile([128, S_FULL], BF16)  # V^T (p0:64) / K^T (p64:)
            krep = const.tile([64, S_FULL], BF16)  # K^T replica on p0:64
            qtpair = const.tile([128, TQ], BF16)  # Q^T on both halves
            vp_sb = const.tile([128, KT, E + 1], BF16)  # V' = [V | 1]
            # only the ones column needs a memset; vtrans fills the rest
            nc.gpsimd.memset(vp_sb[:, :, E:E + 1], 1.0)

            # pull the EXP activation-table load into the DMA window
            warm_sb = const.tile([128, 1], F32)
            nc.scalar.activation(
                warm_sb, bkv_sb, mybir.ActivationFunctionType.Exp, scale=0.0
            )

            with tc.tile_pool(name="psP", bufs=3, space="PSUM") as psP:
                # keep the PE HAM clock-gate warm while the x stream lands;
                # identity transposes start as soon as make_identity is done
                # (no dependency on the weight/x DMAs), then a few weight
                # matmuls bridge the gap until the first x chunk arrives
                for i in range(N_WARM):
                    wm = psP.tile([64, 64], BF16, tag="spair", name=f"wm{i}")
                    nc.tensor.transpose(wm, ident_bf, ident_bf)
                for i in range(8):
                    wm = psP.tile([128, 128], F32, tag="spair",
                                  name=f"wmb{i}")
                    nc.tensor.matmul(
                        wm, wkv_sb[:, i % WT, :], wkv_sb[:, (i + 1) % WT, :],
                        start=True, stop=True,
                    )

                def emit_vtrans_mm(ch, j, vt_ps):
                    nc.tensor.transpose(
                        vt_ps[:, j, :],
                        kv_sb[0:64, ts(4 * ch + j, 128)], ident_bf
                    )

                def emit_chunk(ch):
                    """KV (+Q) projection for chunk ch, interleaved with the
                    previous chunk's V' transposes so consecutive PE matmuls
                    never accumulate into the same PSUM bank (same-bank
                    accumulation serializes stream+drain at ~2x cost)."""
                    ps = psP.tile([128, 512], F32, tag="spair",
                                  name=f"pskv{ch}")
                    psq = None
                    if ch < QC:
                        psq = psP.tile([128, 512], F32, tag="spair",
                                       name=f"psq{ch}")
                    vt_ps = None
                    if ch > 0:
                        vt_ps = psP.tile([128, 4, E], BF16, tag="spair",
                                         name=f"vtps{ch - 1}")
                    for t in range(WT):
                        nc.tensor.matmul(
                            ps,
                            wkv_sb[:, t, :],
                            xt_sb[:, t, ts(ch, 512)],
                            start=(t == 0),
                            stop=(t == WT - 1),
                        )
                        if psq is not None:
                            nc.tensor.matmul(
                                psq,
                                wqq_sb[:, t, :],
                                xt_sb[:, t, ts(ch, 512)],
                                start=(t == 0),
                                stop=(t == WT - 1),
                            )
                        if vt_ps is not None:
                            emit_vtrans_mm(ch - 1, t, vt_ps)
                    # bias add + PSUM->SBUF moves split across both engines
                    nc.scalar.activation(
                        kv_sb[:, ts(ch, 512)], ps,
                        mybir.ActivationFunctionType.Identity,
                        bias=bkv_sb,
                    )
                    # K replica on the fast HWDGE queue (SWDGE takes ~6us to
                    # land this partition-shifted copy)
                    nc.sync.dma_start(
                        out=krep[:, ts(ch, 512)], in_=kv_sb[64:128, ts(ch, 512)]
                    )
                    if psq is not None:
                        nc.vector.tensor_scalar_add(
                            qtpair[:, ts(ch, 512)], psq, bqq_sb
                        )
                    if vt_ps is not None:
                        nc.vector.tensor_copy(
                            vp_sb[:, 4 * (ch - 1):4 * ch, 0:E], vt_ps
                        )

                def emit_vtrans_last():
                    vt_ps = psP.tile([128, 4, E], BF16, tag="spair",
                                     name=f"vtps{NCH - 1}")
                    for j in range(4):
                        emit_vtrans_mm(NCH - 1, j, vt_ps)
                    nc.vector.tensor_copy(
                        vp_sb[:, 4 * (NCH - 1):4 * NCH, 0:E], vt_ps
                    )

                state = {"pending": [], "zps": None, "prev_zps": None,
                         "it": 0}

                def emit_copies(qc_done, zps):
                    # evacuate the finished accumulators; host does the divide
                    zpA, zpB = zps
                    zsbA = zsbp.tile([E + 1, 512], F32, tag="zsb",
                                     name=f"zsbA{qc_done}")
                    zsbB = zsbp.tile([E + 1, 512], F32, tag="zsb",
                                     name=f"zsbB{qc_done}")
                    nc.scalar.activation(
                        zsbA, zpA, mybir.ActivationFunctionType.Copy
                    )
                    nc.vector.tensor_copy(zsbB, zpB)
                    nc.gpsimd.dma_start(out=y[0, qc_done], in_=zsbA)
                    nc.gpsimd.dma_start(out=y[1, qc_done], in_=zsbB)

                def emit_av(pp_, pka, pkb, zps):
                    zpA, zpB = zps
                    nc.tensor.matmul(
                        zpA, vp_sb[0:64, pka, :], pp_[0:64, 0:512],
                        start=(pka == 0), stop=False,
                    )
                    nc.tensor.matmul(
                        zpB, vp_sb[64:128, pka, :], pp_[64:128, 0:512],
                        start=(pka == 0), stop=False,
                    )
                    nc.tensor.matmul(
                        zpA, vp_sb[0:64, pkb, :], pp_[0:64, 512:1024],
                        start=False, stop=(pkb == KT - 1),
                    )
                    nc.tensor.matmul(
                        zpB, vp_sb[64:128, pkb, :], pp_[64:128, 512:1024],
                        start=False, stop=(pkb == KT - 1),
                    )

                def emit_iter(qc, kp):
                    pending = state["pending"]
                    # drain deferred AVs: keep PEND-1 entries before the
                    # append, one fewer at kp==1 so the previous chunk's
                    # accumulators finish early
                    n_keep = PEND - 2 if kp == 1 else PEND - 1
                    while len(pending) > n_keep:
                        emit_av(*pending.pop(0))
                    if kp == 2 and state["prev_zps"] is not None:
                        emit_copies(qc - 1, state["prev_zps"])
                        state["prev_zps"] = None
                    ka, kb = 2 * kp, 2 * kp + 1
                    sp = psP.tile(
                        [128, 1024], F32, tag="spair", name=f"sp{qc}_{kp}"
                    )
                    nc.tensor.matmul(
                        sp[:, 0:512],
                        krep[:, ts(ka, 128)],
                        qtpair[0:64, ts(qc, 512)],
                        start=True,
                        stop=True,
                    )
                    nc.tensor.matmul(
                        sp[:, 512:1024],
                        kv_sb[64:128, ts(kb, 128)],
                        qtpair[64:128, ts(qc, 512)],
                        start=True,
                        stop=True,
                    )
                    p_sb = ppool.tile(
                        [128, 1024], BF16, tag="p", name=f"p{qc}_{kp}"
                    )
                    # whole pairs alternate between the two exp engines
                    if state["it"] % 2 == 0:
                        nc.scalar.activation(
                            p_sb, sp, mybir.ActivationFunctionType.Exp,
                            scale=SCALE,
                        )
                    else:
                        nc.vector.tensor_scalar(
                            out=p_sb[:, :].bitcast(I16),
                            in0=sp,
                            scalar1=SCH_A,
                            scalar2=SCH_B,
                            op0=mybir.AluOpType.mult,
                            op1=mybir.AluOpType.add,
                        )
                    state["it"] += 1
                    pending.append((p_sb, ka, kb, state["zps"]))

                def roll_qc(qc_done):
                    state["prev_zps"] = state["zps"]
                    if qc_done < QC - 1:
                        new_zps(qc_done + 1)

                def new_zps(qc):
                    state["zps"] = (
                        psP.tile([E + 1, 512], F32, tag="zacc", bufs=2,
                                 name=f"zaccA{qc}"),
                        psP.tile([E + 1, 512], F32, tag="zacc", bufs=2,
                                 name=f"zaccB{qc}"),
                    )

                # prologue chunk 0: projections only (no keys ready yet)
                emit_chunk(0)

                # qc0 sweep chases the DMA stream: chunk ch's projection is
                # followed by the two qc0 iterations over chunk ch-1's keys
                new_zps(0)
                for ch in range(1, NCH):
                    if ch in (3, 5):  # x blocks 2-3 just-in-time on sync
                        emit_xblk(ch // 2 + 1)
                    emit_chunk(ch)
                    emit_iter(0, 2 * (ch - 1))
                    emit_iter(0, 2 * (ch - 1) + 1)
                emit_vtrans_last()
                for kp in range(2 * (NCH - 1), KP):
                    emit_iter(0, kp)
                roll_qc(0)

                for qc in range(1, QC):
                    for kp in range(KP):
                        emit_iter(qc, kp)
                    roll_qc(qc)

                # tail: drain the last AVs and evacuate the final chunk
                for args in state["pending"]:
                    emit_av(*args)
                state["pending"] = []
                emit_copies(QC - 1, state["prev_zps"])
    nc.compile()
    return nc


def get_nc():
    if "nc" not in _NC_CACHE:
        _NC_CACHE["nc"] = build_nc()
    return _NC_CACHE["nc"]


def make_in_maps(x, Wq, bq, Wk, bk, Wv, bv):
    x = np.asarray(x, dtype=np.float32)
    Wq = np.asarray(Wq, dtype=np.float32)
    Wk = np.asarray(Wk, dtype=np.float32)
    Wv = np.asarray(Wv, dtype=np.float32)
    bq = np.asarray(bq, dtype=np.float32)
    bk = np.asarray(bk, dtype=np.float32)
    bv = np.asarray(bv, dtype=np.float32)

    def rearrange_w(w):  # [W, 128] -> [128, WT, 128] with p = w % 128
        return np.ascontiguousarray(
            w.reshape(WT, 128, 128).transpose(1, 0, 2)
        )

    wkv_host = rearrange_w(
        np.concatenate([Wv.T, Wk.T], axis=1).astype(NP_BF16)
    )
    wqq_host = rearrange_w(
        np.concatenate([Wq.T, Wq.T], axis=1).astype(NP_BF16)
    )
    bkv_host = np.ascontiguousarray(
        np.concatenate([bv, bk]).reshape(128, 1)
    ).astype(np.float32)
    bqq_host = np.ascontiguousarray(
        np.concatenate([bq, bq]).reshape(128, 1)
    ).astype(np.float32)

    in_maps = []
    for c in range(8):
        b, h = c // 2, c % 2
        xT_b = np.asarray(x[b].T, dtype=NP_BF16)
        if h == 1:  # put this core's query half into columns [0, 2048)
            xT_b = np.concatenate([xT_b[:, TQ:], xT_b[:, :TQ]], axis=1)
        in_maps.append(
            {
                "xT": np.ascontiguousarray(xT_b),
                "wqq": wqq_host,
                "wkv": wkv_host,
                "bqq": bqq_host,
                "bkv": bkv_host,
            }
        )
    return in_maps


def assemble(results):
    out = np.empty((B, S_FULL, E), dtype=np.float32)
    for c in range(8):
        b, h = c // 2, c % 2
        z = results[c]["y"]  # [2, QC, E+1, 512]
        z = z[0] + z[1]  # [QC, E+1, 512]
        o = z[:, 0:E, :] / z[:, E:E + 1, :]  # normalize
        o = o.transpose(0, 2, 1).reshape(TQ, E)  # [QC,E,512] -> [2048, E]
        out[b, h * TQ:(h + 1) * TQ, :] = o
    return out


def kernel(x, Wq, bq, Wk, bk, Wv, bv, **_unused):
    in_maps = make_in_maps(x, Wq, bq, Wk, bk, Wv, bv)
    nc = get_nc()
    res = run_bass_kernel_spmd(nc, in_maps, core_ids=list(range(8)))
    return assemble(res.results)
